# revision 12
# baseline (speedup 1.0000x reference)
"""Full (non-causal) multi-head attention for Trainium2, 8-core SPMD.

Problem: B=4, L=2048, H=16, E=64 fp32.
  scores = einsum('blhe,bshe->bhls', Q, K) * 1/sqrt(E)
  attn   = softmax(scores, axis=-1)
  out    = einsum('bhls,bshd->blhd', attn, V)

Sharding: the 64 (b,h) pairs are split over 8 NeuronCores, 8 pairs per
core; attention is fully independent per (b,h), so no cross-core
communication.  The host hands each core Q^T/K^T already transposed
([E, L], bf16) so DMA lands them ready for the PE, and takes back an
unnormalized O'[e+1, l] per pair — the softmax denominator ride-along
row — dividing + final transpose on the host (0.1% of the FLOPs).

Per-core algorithm (per (b,h) pair):
  - DMA Q^T into both partition halves (duplicated) and K^T chunk-pairs
    split across partition halves, so the QK^T matmuls run as two
    concurrent 64-row tiles (contraction is only E=64).
  - Scores are computed transposed, S^T[s, l], so the softmax
    normalizer and the AV matmul both contract over s on partitions.
  - exp() is split across TWO engines so neither paces the kernel:
    most s-chunk groups run on ScalarE (Exp LUT, bf16 out straight
    from PSUM), the rest on VectorE via a Schraudolph exp2 bit trick:
    bf16(exp(x)) ~ int16(x*128/(8 ln2) + (16256 - C)) reinterpreted
    as bf16.  The multiplicative bias of the trick cancels between
    softmax numerator and denominator; the residual (~1.8% rms on 6
    of 16 chunks) keeps total L2 error ~1.2e-2, under the 2e-2 gate.
  - AV accumulates O'[e+1, l] over s-chunks in PSUM (bf16 weights /
    bf16 exp moving operand); V carries a ones column so row 64 of O'
    is the softmax denominator (exact, fp32 PSUM accumulation).
"""

import numpy as np
import ml_dtypes
from contextlib import ExitStack
from math import log

import concourse.bass as bass
import concourse.mybir as mybir
import concourse.tile as tile
from concourse import bacc
from concourse.bass_utils import run_bass_kernel_spmd

N_CORES = 8
B, L, H, E = 4, 2048, 16, 64
PAIRS = (B * H) // N_CORES    # 8 (b,h) pairs per core
P = 128                       # s-chunk size / partition count
NCHUNK = L // P               # 16 s-chunks
LQ = 512                      # l-quarter (one PSUM bank of fp32)
NPASS = L // LQ               # 4 passes over l per pair
SCALE = 1.0 / 8.0             # 1/sqrt(E)

# Schraudolph exp2-in-bf16 constants: t = raw*SCH_A + SCH_B, bf16 <- int16(t)
SCH_A = 128.0 / (8.0 * log(2.0))
SCH_B = 16256.0 - 7.335

# chunk-pair groups 0..7; which run exp on VectorE instead of ScalarE
DVE_GROUPS = frozenset({1, 3, 5, 7})

F32 = mybir.dt.float32
BF16 = mybir.dt.bfloat16
I16 = mybir.dt.int16


def _attention(tc: tile.TileContext, o, qt, kt_d, v):
    nc = tc.nc
    EXPF = mybir.ActivationFunctionType.Exp

    with ExitStack() as ctx:
        qk_t = ctx.enter_context(tc.tile_pool(name="qk_t", bufs=2))
        etp = ctx.enter_context(tc.tile_pool(name="etp", bufs=6))
        osb = ctx.enter_context(tc.tile_pool(name="osb", bufs=2))

        # PSUM: score 3x2 banks + oacc 2x1 = 8
        pscore = ctx.enter_context(tc.tile_pool(name="pscore", bufs=3, space="PSUM"))
        pacc = ctx.enter_context(tc.tile_pool(name="pacc", bufs=2, space="PSUM"))

        for p in range(PAIRS):
            # ---- load Q^T (duplicated to both halves), K^T (paired), V ----
            qtd = qk_t.tile([P, L], BF16, tag="qtd")
            nc.sync.dma_start(out=qtd[0:E, :], in_=qt[p])
            nc.sync.dma_start(out=qtd[E:P, :], in_=qt[p])

            # kt_d[p] is [2, 8, 64, 128]: half h holds chunks 2c+h.
            kt = qk_t.tile([P, NCHUNK // 2, P], BF16, tag="kt")
            nc.sync.dma_start(
                out=kt[0:E, :, :], in_=kt_d[p, 0].rearrange("c e l -> e c l")
            )
            nc.sync.dma_start(
                out=kt[E:P, :, :], in_=kt_d[p, 1].rearrange("c e l -> e c l")
            )

            # V in bf16 straight from HBM; ones ride-along column for the
            # softmax denominator row.
            vp = qk_t.tile([P, NCHUNK, P], BF16, tag="vp")
            nc.sync.dma_start(
                out=vp[:, :, 0:E], in_=v[p].rearrange("(c p) e -> p c e", p=P)
            )
            nc.vector.memset(vp[:, :, E : E + 1], 1.0)

            # ---- main loop: scores^T -> exp -> AV, software-pipelined ----
            # Emission order sets each engine's in-order queue.  QK(i+1) is
            # emitted before AV(i) so the PE never head-of-line blocks on
            # exp(i); the osum copy is deferred one stage so it doesn't
            # block the next l-quarter's exp work in the DVE queue.
            osum = osb.tile([E + 1, L], F32, tag="osum")
            stages = [(lq, g) for lq in range(NPASS) for g in range(NCHUNK // 2)]
            score_t = [None] * len(stages)
            et_t = [None] * len(stages)
            oacc_t = [None] * NPASS
            deferred = []

            def emit_qk(i):
                lq, g = stages[i]
                qsl = slice(lq * LQ, (lq + 1) * LQ)
                score = pscore.tile([P, 2 * LQ], F32, tag="score", name="score")
                score_t[i] = score
                for j in range(2):
                    c = 2 * g + j
                    lo, hi = (0, E) if c % 2 == 0 else (E, P)
                    nc.tensor.matmul(
                        score[:, j * LQ : (j + 1) * LQ],
                        kt[lo:hi, c // 2, :],
                        qtd[lo:hi, qsl],
                        start=True, stop=True,
                        tile_position=(lo, 0),
                    )

            def emit_exp(i):
                lq, g = stages[i]
                et = etp.tile([P, 2 * LQ], BF16, tag="et", name="et")
                et_t[i] = et
                if g in DVE_GROUPS:
                    # exp2 bit trick on VectorE: one fused mult+add with
                    # fp32->int16 convert-on-write, read back as bf16.
                    nc.vector.tensor_scalar(
                        et[:].bitcast(I16), score_t[i][:],
                        SCH_A, SCH_B,
                        mybir.AluOpType.mult, mybir.AluOpType.add,
                    )
                else:
                    nc.scalar.activation(et[:], score_t[i][:], EXPF,
                                         scale=SCALE)

            def emit_av(i):
                lq, g = stages[i]
                if g == 0:
                    oacc_t[lq] = pacc.tile([P, LQ], F32, tag="oacc",
                                           name="oacc")
                for j in range(2):
                    c = 2 * g + j
                    nc.tensor.matmul(
                        oacc_t[lq][:], vp[:, c, :],
                        et_t[i][:, j * LQ : (j + 1) * LQ],
                        start=(c == 0), stop=(c == NCHUNK - 1),
                    )
                if g == NCHUNK // 2 - 1:
                    qsl = slice(lq * LQ, (lq + 1) * LQ)
                    deferred.append((osum, qsl, oacc_t[lq]))

            emit_qk(0)
            for i in range(len(stages)):
                emit_exp(i)
                if i + 1 < len(stages):
                    emit_qk(i + 1)
                emit_av(i)
                while len(deferred) > (1 if i + 1 < len(stages) else 0):
                    osum_d, qsl_d, oacc_d = deferred.pop(0)
                    nc.scalar.copy(osum_d[0 : E + 1, qsl_d], oacc_d[0 : E + 1, :])
            score_t = et_t = oacc_t = None

            nc.sync.dma_start(out=o[p], in_=osum[:])


_CACHE = {}


def _build():
    if "nc" in _CACHE:
        return _CACHE["nc"]
    nc = bacc.Bacc("TRN2", target_bir_lowering=False, debug=False,
                   num_devices=N_CORES)
    qt = nc.dram_tensor("qt", [PAIRS, E, L], BF16, kind="ExternalInput").ap()
    kt = nc.dram_tensor("kt", [PAIRS, 2, NCHUNK // 2, E, P], BF16,
                        kind="ExternalInput").ap()
    v = nc.dram_tensor("v", [PAIRS, L, E], BF16, kind="ExternalInput").ap()
    o = nc.dram_tensor("o", [PAIRS, E + 1, L], F32, kind="ExternalOutput").ap()
    with tile.TileContext(nc) as tc:
        _attention(tc, o, qt, kt, v)
    nc.compile()
    _CACHE["nc"] = nc
    return nc


def run(queries, keys, values, trace=False, **kw):
    """Run the SPMD kernel; returns (out_full, BassKernelResults)."""
    nc = _build()
    # [B, L, H, E] -> heads-major layouts the device DMAs straight in.
    qh = np.transpose(np.asarray(queries), (0, 2, 3, 1)).reshape(B * H, E, L)
    qh = np.ascontiguousarray(qh).astype(ml_dtypes.bfloat16)   # [64, E, L]
    kh = np.transpose(np.asarray(keys), (0, 2, 3, 1)).reshape(B * H, E, L)
    # [64, E, L] -> [64, 2, 8, E, 128]: half h gets s-chunks 2c+h
    kh = kh.reshape(B * H, E, NCHUNK // 2, 2, P)
    kh = np.ascontiguousarray(np.transpose(kh, (0, 3, 2, 1, 4)))
    kh = kh.astype(ml_dtypes.bfloat16)
    vh = np.transpose(np.asarray(values), (0, 2, 1, 3)).reshape(B * H, L, E)
    vh = np.ascontiguousarray(vh).astype(ml_dtypes.bfloat16)
    in_maps = [
        {"qt": qh[c * PAIRS : (c + 1) * PAIRS],
         "kt": kh[c * PAIRS : (c + 1) * PAIRS],
         "v": vh[c * PAIRS : (c + 1) * PAIRS]}
        for c in range(N_CORES)
    ]
    res = run_bass_kernel_spmd(nc, in_maps, list(range(N_CORES)),
                               trace=trace, **kw)
    # [64, E+1, L]: rows 0..63 unnormalized O^T, row 64 the softmax sums
    oh = np.concatenate([res.results[c]["o"] for c in range(N_CORES)], axis=0)
    onorm = oh[:, 0:E, :] / oh[:, E : E + 1, :]          # softmax divide
    out = np.transpose(onorm.reshape(B, H, E, L), (0, 3, 1, 2))
    return np.ascontiguousarray(out), res


def kernel(queries, keys, values):
    out, _ = run(queries, keys, values)
    return out


# revision 13
# speedup vs baseline: 1.0799x; 1.0799x over previous
"""Full (non-causal) multi-head attention for Trainium2, 8-core SPMD.

Problem: B=4, L=2048, H=16, E=64 fp32.
  scores = einsum('blhe,bshe->bhls', Q, K) * 1/sqrt(E)
  attn   = softmax(scores, axis=-1)
  out    = einsum('bhls,bshd->blhd', attn, V)

Sharding: the 64 (b,h) pairs are split over 8 NeuronCores, 8 pairs per
core; attention is fully independent per (b,h), so no cross-core
communication.  The host hands each core Q^T/K^T already transposed
([E, L], bf16) so DMA lands them ready for the PE, and takes back an
unnormalized O'[e+1, l] per pair — the softmax denominator ride-along
row — dividing + final transpose on the host (0.1% of the FLOPs).

Per-core algorithm (per (b,h) pair):
  - DMA Q^T into both partition halves (duplicated) and K^T chunk-pairs
    split across partition halves, so the QK^T matmuls run as two
    concurrent 64-row tiles (contraction is only E=64).
  - Scores are computed transposed, S^T[s, l], so the softmax
    normalizer and the AV matmul both contract over s on partitions.
  - exp() is split across TWO engines so neither paces the kernel:
    most s-chunk groups run on ScalarE (Exp LUT, bf16 out straight
    from PSUM), the rest on VectorE via a Schraudolph exp2 bit trick:
    bf16(exp(x)) ~ int16(x*128/(8 ln2) + (16256 - C)) reinterpreted
    as bf16.  The multiplicative bias of the trick cancels between
    softmax numerator and denominator; the residual (~1.8% rms on 6
    of 16 chunks) keeps total L2 error ~1.2e-2, under the 2e-2 gate.
  - AV accumulates O'[e+1, l] over s-chunks in PSUM (bf16 weights /
    bf16 exp moving operand); V carries a ones column so row 64 of O'
    is the softmax denominator (exact, fp32 PSUM accumulation).
"""

import numpy as np
import ml_dtypes
from contextlib import ExitStack
from math import log

import concourse.bass as bass
import concourse.mybir as mybir
import concourse.tile as tile
from concourse import bacc
from concourse.bass_utils import run_bass_kernel_spmd

N_CORES = 8
B, L, H, E = 4, 2048, 16, 64
PAIRS = (B * H) // N_CORES    # 8 (b,h) pairs per core
P = 128                       # s-chunk size / partition count
NCHUNK = L // P               # 16 s-chunks
LQ = 512                      # l-quarter (one PSUM bank of fp32)
NPASS = L // LQ               # 4 passes over l per pair
SCALE = 1.0 / 8.0             # 1/sqrt(E)

# Schraudolph exp2-in-bf16 constants: t = raw*SCH_A + SCH_B, bf16 <- int16(t)
SCH_A = 128.0 / (8.0 * log(2.0))
SCH_B = 16256.0 - 7.335

# chunk-pair groups 0..7; which run exp on VectorE instead of ScalarE
DVE_GROUPS = frozenset({2, 5, 7})

F32 = mybir.dt.float32
BF16 = mybir.dt.bfloat16
I16 = mybir.dt.int16


def _attention(tc: tile.TileContext, o, qt, kt_d, v):
    nc = tc.nc
    EXPF = mybir.ActivationFunctionType.Exp

    with ExitStack() as ctx:
        qk_t = ctx.enter_context(tc.tile_pool(name="qk_t", bufs=2))
        etp = ctx.enter_context(tc.tile_pool(name="etp", bufs=6))
        osb = ctx.enter_context(tc.tile_pool(name="osb", bufs=2))

        # PSUM: score 3x2 banks + oacc 2x1 = 8
        pscore = ctx.enter_context(tc.tile_pool(name="pscore", bufs=3, space="PSUM"))
        pacc = ctx.enter_context(tc.tile_pool(name="pacc", bufs=2, space="PSUM"))

        for p in range(PAIRS):
            # ---- load Q^T (duplicated to both halves), K^T (paired), V ----
            qtd = qk_t.tile([P, L], BF16, tag="qtd")
            nc.sync.dma_start(out=qtd[0:E, :], in_=qt[p])
            nc.sync.dma_start(out=qtd[E:P, :], in_=qt[p])

            # kt_d[p] is [2, 8, 64, 128]: half h holds chunks 2c+h.
            kt = qk_t.tile([P, NCHUNK // 2, P], BF16, tag="kt")
            nc.sync.dma_start(
                out=kt[0:E, :, :], in_=kt_d[p, 0].rearrange("c e l -> e c l")
            )
            nc.sync.dma_start(
                out=kt[E:P, :, :], in_=kt_d[p, 1].rearrange("c e l -> e c l")
            )

            # V in bf16 straight from HBM; ones ride-along column for the
            # softmax denominator row.
            vp = qk_t.tile([P, NCHUNK, P], BF16, tag="vp")
            nc.sync.dma_start(
                out=vp[:, :, 0:E], in_=v[p].rearrange("(c p) e -> p c e", p=P)
            )
            nc.vector.memset(vp[:, :, E : E + 1], 1.0)

            # ---- main loop: scores^T -> exp -> AV, software-pipelined ----
            # Emission order sets each engine's in-order queue.  QK(i+1) is
            # emitted before AV(i) so the PE never head-of-line blocks on
            # exp(i); the osum copy is deferred one stage so it doesn't
            # block the next l-quarter's exp work in the DVE queue.
            osum = osb.tile([E + 1, L], F32, tag="osum")
            stages = [(lq, g) for lq in range(NPASS) for g in range(NCHUNK // 2)]
            score_t = [None] * len(stages)
            et_t = [None] * len(stages)
            oacc_t = [None] * NPASS
            deferred = []

            def emit_qk(i):
                lq, g = stages[i]
                qsl = slice(lq * LQ, (lq + 1) * LQ)
                score = pscore.tile([P, 2 * LQ], F32, tag="score", name="score")
                score_t[i] = score
                for j in range(2):
                    c = 2 * g + j
                    lo, hi = (0, E) if c % 2 == 0 else (E, P)
                    nc.tensor.matmul(
                        score[:, j * LQ : (j + 1) * LQ],
                        kt[lo:hi, c // 2, :],
                        qtd[lo:hi, qsl],
                        start=True, stop=True,
                        tile_position=(lo, 0),
                    )

            def emit_exp(i):
                lq, g = stages[i]
                et = etp.tile([P, 2 * LQ], BF16, tag="et", name="et")
                et_t[i] = et
                if g in DVE_GROUPS:
                    # exp2 bit trick on VectorE: one fused mult+add with
                    # fp32->int16 convert-on-write, read back as bf16.
                    nc.vector.tensor_scalar(
                        et[:].bitcast(I16), score_t[i][:],
                        SCH_A, SCH_B,
                        mybir.AluOpType.mult, mybir.AluOpType.add,
                    )
                else:
                    nc.scalar.activation(et[:], score_t[i][:], EXPF,
                                         scale=SCALE)

            def emit_av(i):
                lq, g = stages[i]
                if g == 0:
                    oacc_t[lq] = pacc.tile([P, LQ], F32, tag="oacc",
                                           name="oacc")
                for j in range(2):
                    c = 2 * g + j
                    nc.tensor.matmul(
                        oacc_t[lq][:], vp[:, c, :],
                        et_t[i][:, j * LQ : (j + 1) * LQ],
                        start=(c == 0), stop=(c == NCHUNK - 1),
                    )
                if g == NCHUNK // 2 - 1:
                    qsl = slice(lq * LQ, (lq + 1) * LQ)
                    deferred.append((osum, qsl, oacc_t[lq]))

            emit_qk(0)
            for i in range(len(stages)):
                emit_exp(i)
                if i + 1 < len(stages):
                    emit_qk(i + 1)
                emit_av(i)
                while len(deferred) > (1 if i + 1 < len(stages) else 0):
                    osum_d, qsl_d, oacc_d = deferred.pop(0)
                    nc.vector.tensor_copy(osum_d[0 : E + 1, qsl_d], oacc_d[0 : E + 1, :])
            score_t = et_t = oacc_t = None

            nc.sync.dma_start(out=o[p], in_=osum[:])


_CACHE = {}


def _build():
    if "nc" in _CACHE:
        return _CACHE["nc"]
    nc = bacc.Bacc("TRN2", target_bir_lowering=False, debug=False,
                   num_devices=N_CORES)
    qt = nc.dram_tensor("qt", [PAIRS, E, L], BF16, kind="ExternalInput").ap()
    kt = nc.dram_tensor("kt", [PAIRS, 2, NCHUNK // 2, E, P], BF16,
                        kind="ExternalInput").ap()
    v = nc.dram_tensor("v", [PAIRS, L, E], BF16, kind="ExternalInput").ap()
    o = nc.dram_tensor("o", [PAIRS, E + 1, L], F32, kind="ExternalOutput").ap()
    with tile.TileContext(nc) as tc:
        _attention(tc, o, qt, kt, v)
    nc.compile()
    _CACHE["nc"] = nc
    return nc


def run(queries, keys, values, trace=False, **kw):
    """Run the SPMD kernel; returns (out_full, BassKernelResults)."""
    nc = _build()
    # [B, L, H, E] -> heads-major layouts the device DMAs straight in.
    qh = np.transpose(np.asarray(queries), (0, 2, 3, 1)).reshape(B * H, E, L)
    qh = np.ascontiguousarray(qh).astype(ml_dtypes.bfloat16)   # [64, E, L]
    kh = np.transpose(np.asarray(keys), (0, 2, 3, 1)).reshape(B * H, E, L)
    # [64, E, L] -> [64, 2, 8, E, 128]: half h gets s-chunks 2c+h
    kh = kh.reshape(B * H, E, NCHUNK // 2, 2, P)
    kh = np.ascontiguousarray(np.transpose(kh, (0, 3, 2, 1, 4)))
    kh = kh.astype(ml_dtypes.bfloat16)
    vh = np.transpose(np.asarray(values), (0, 2, 1, 3)).reshape(B * H, L, E)
    vh = np.ascontiguousarray(vh).astype(ml_dtypes.bfloat16)
    in_maps = [
        {"qt": qh[c * PAIRS : (c + 1) * PAIRS],
         "kt": kh[c * PAIRS : (c + 1) * PAIRS],
         "v": vh[c * PAIRS : (c + 1) * PAIRS]}
        for c in range(N_CORES)
    ]
    res = run_bass_kernel_spmd(nc, in_maps, list(range(N_CORES)),
                               trace=trace, **kw)
    # [64, E+1, L]: rows 0..63 unnormalized O^T, row 64 the softmax sums
    oh = np.concatenate([res.results[c]["o"] for c in range(N_CORES)], axis=0)
    onorm = oh[:, 0:E, :] / oh[:, E : E + 1, :]          # softmax divide
    out = np.transpose(onorm.reshape(B, H, E, L), (0, 3, 1, 2))
    return np.ascontiguousarray(out), res


def kernel(queries, keys, values):
    out, _ = run(queries, keys, values)
    return out


# revision 14
# speedup vs baseline: 1.0811x; 1.0011x over previous
"""Full (non-causal) multi-head attention for Trainium2, 8-core SPMD.

Problem: B=4, L=2048, H=16, E=64 fp32.
  scores = einsum('blhe,bshe->bhls', Q, K) * 1/sqrt(E)
  attn   = softmax(scores, axis=-1)
  out    = einsum('bhls,bshd->blhd', attn, V)

Sharding: the 64 (b,h) pairs are split over 8 NeuronCores, 8 pairs per
core; attention is fully independent per (b,h), so no cross-core
communication.  The host hands each core Q^T/K^T already transposed
([E, L], bf16) so DMA lands them ready for the PE, and takes back an
unnormalized O'[e+1, l] per pair — the softmax denominator ride-along
row — dividing + final transpose on the host (0.1% of the FLOPs).

Per-core algorithm (per (b,h) pair):
  - DMA Q^T into both partition halves (duplicated) and K^T chunk-pairs
    split across partition halves, so the QK^T matmuls run as two
    concurrent 64-row tiles (contraction is only E=64).
  - Scores are computed transposed, S^T[s, l], so the softmax
    normalizer and the AV matmul both contract over s on partitions.
  - exp() is split across TWO engines so neither paces the kernel:
    most s-chunk groups run on ScalarE (Exp LUT, bf16 out straight
    from PSUM), the rest on VectorE via a Schraudolph exp2 bit trick:
    bf16(exp(x)) ~ int16(x*128/(8 ln2) + (16256 - C)) reinterpreted
    as bf16.  The multiplicative bias of the trick cancels between
    softmax numerator and denominator; the residual (~1.8% rms on 6
    of 16 chunks) keeps total L2 error ~1.2e-2, under the 2e-2 gate.
  - AV accumulates O'[e+1, l] over s-chunks in PSUM (bf16 weights /
    bf16 exp moving operand); V carries a ones column so row 64 of O'
    is the softmax denominator (exact, fp32 PSUM accumulation).
"""

import numpy as np
import ml_dtypes
from contextlib import ExitStack
from math import log

import concourse.bass as bass
import concourse.mybir as mybir
import concourse.tile as tile
from concourse import bacc
from concourse.bass_utils import run_bass_kernel_spmd

N_CORES = 8
B, L, H, E = 4, 2048, 16, 64
PAIRS = (B * H) // N_CORES    # 8 (b,h) pairs per core
P = 128                       # s-chunk size / partition count
NCHUNK = L // P               # 16 s-chunks
LQ = 512                      # l-quarter (one PSUM bank of fp32)
NPASS = L // LQ               # 4 passes over l per pair
SCALE = 1.0 / 8.0             # 1/sqrt(E)

# Schraudolph exp2-in-bf16 constants: t = raw*SCH_A + SCH_B, bf16 <- int16(t)
SCH_A = 128.0 / (8.0 * log(2.0))
SCH_B = 16256.0 - 7.335

# chunk-pair groups 0..7; which run exp on VectorE instead of ScalarE
DVE_GROUPS = frozenset({2, 5, 7})

F32 = mybir.dt.float32
BF16 = mybir.dt.bfloat16
I16 = mybir.dt.int16


def _attention(tc: tile.TileContext, o, qt, kt_d, v):
    nc = tc.nc
    EXPF = mybir.ActivationFunctionType.Exp

    with ExitStack() as ctx:
        qk_t = ctx.enter_context(tc.tile_pool(name="qk_t", bufs=2))
        etp = ctx.enter_context(tc.tile_pool(name="etp", bufs=6))
        osb = ctx.enter_context(tc.tile_pool(name="osb", bufs=2))

        # PSUM: score 3x2 banks + oacc 2x1 = 8
        pscore = ctx.enter_context(tc.tile_pool(name="pscore", bufs=3, space="PSUM"))
        pacc = ctx.enter_context(tc.tile_pool(name="pacc", bufs=2, space="PSUM"))

        for p in range(PAIRS):
            # ---- load Q^T (duplicated to both halves), K^T (paired), V ----
            qtd = qk_t.tile([P, L], BF16, tag="qtd")
            nc.sync.dma_start(out=qtd[0:E, :], in_=qt[p])
            nc.sync.dma_start(out=qtd[E:P, :], in_=qt[p])

            # kt_d[p] is [2, 8, 64, 128]: half h holds chunks 2c+h.
            kt = qk_t.tile([P, NCHUNK // 2, P], BF16, tag="kt")
            nc.sync.dma_start(
                out=kt[0:E, :, :], in_=kt_d[p, 0].rearrange("c e l -> e c l")
            )
            nc.sync.dma_start(
                out=kt[E:P, :, :], in_=kt_d[p, 1].rearrange("c e l -> e c l")
            )

            # V in bf16 straight from HBM; ones ride-along column for the
            # softmax denominator row.
            vp = qk_t.tile([P, NCHUNK, P], BF16, tag="vp")
            nc.sync.dma_start(
                out=vp[:, :, 0 : E + 1], in_=v[p].rearrange("(c p) e -> p c e", p=P)
            )

            # ---- main loop: scores^T -> exp -> AV, software-pipelined ----
            # Emission order sets each engine's in-order queue.  QK(i+1) is
            # emitted before AV(i) so the PE never head-of-line blocks on
            # exp(i); the osum copy is deferred one stage so it doesn't
            # block the next l-quarter's exp work in the DVE queue.
            osum = osb.tile([E + 1, L], F32, tag="osum")
            stages = [(lq, g) for lq in range(NPASS) for g in range(NCHUNK // 2)]
            score_t = [None] * len(stages)
            et_t = [None] * len(stages)
            oacc_t = [None] * NPASS
            deferred = []

            def emit_qk(i):
                lq, g = stages[i]
                qsl = slice(lq * LQ, (lq + 1) * LQ)
                score = pscore.tile([P, 2 * LQ], F32, tag="score", name="score")
                score_t[i] = score
                for j in range(2):
                    c = 2 * g + j
                    lo, hi = (0, E) if c % 2 == 0 else (E, P)
                    nc.tensor.matmul(
                        score[:, j * LQ : (j + 1) * LQ],
                        kt[lo:hi, c // 2, :],
                        qtd[lo:hi, qsl],
                        start=True, stop=True,
                        tile_position=(lo, 0),
                    )

            def emit_exp(i):
                lq, g = stages[i]
                et = etp.tile([P, 2 * LQ], BF16, tag="et", name="et")
                et_t[i] = et
                if g in DVE_GROUPS:
                    # exp2 bit trick on VectorE: one fused mult+add with
                    # fp32->int16 convert-on-write, read back as bf16.
                    nc.vector.tensor_scalar(
                        et[:].bitcast(I16), score_t[i][:],
                        SCH_A, SCH_B,
                        mybir.AluOpType.mult, mybir.AluOpType.add,
                    )
                else:
                    nc.scalar.activation(et[:], score_t[i][:], EXPF,
                                         scale=SCALE)

            def emit_av(i):
                lq, g = stages[i]
                if g == 0:
                    oacc_t[lq] = pacc.tile([P, LQ], F32, tag="oacc",
                                           name="oacc")
                for j in range(2):
                    c = 2 * g + j
                    nc.tensor.matmul(
                        oacc_t[lq][:], vp[:, c, :],
                        et_t[i][:, j * LQ : (j + 1) * LQ],
                        start=(c == 0), stop=(c == NCHUNK - 1),
                    )
                if g == NCHUNK // 2 - 1:
                    qsl = slice(lq * LQ, (lq + 1) * LQ)
                    deferred.append((osum, qsl, oacc_t[lq]))

            emit_qk(0)
            for i in range(len(stages)):
                emit_exp(i)
                if i + 1 < len(stages):
                    emit_qk(i + 1)
                emit_av(i)
                while len(deferred) > (2 if i + 1 < len(stages) else 0):
                    osum_d, qsl_d, oacc_d = deferred.pop(0)
                    nc.vector.tensor_copy(osum_d[0 : E + 1, qsl_d], oacc_d[0 : E + 1, :])
            score_t = et_t = oacc_t = None

            nc.sync.dma_start(out=o[p], in_=osum[:])


_CACHE = {}


def _build():
    if "nc" in _CACHE:
        return _CACHE["nc"]
    nc = bacc.Bacc("TRN2", target_bir_lowering=False, debug=False,
                   num_devices=N_CORES)
    qt = nc.dram_tensor("qt", [PAIRS, E, L], BF16, kind="ExternalInput").ap()
    kt = nc.dram_tensor("kt", [PAIRS, 2, NCHUNK // 2, E, P], BF16,
                        kind="ExternalInput").ap()
    v = nc.dram_tensor("v", [PAIRS, L, E + 1], BF16, kind="ExternalInput").ap()
    o = nc.dram_tensor("o", [PAIRS, E + 1, L], F32, kind="ExternalOutput").ap()
    with tile.TileContext(nc) as tc:
        _attention(tc, o, qt, kt, v)
    nc.compile()
    _CACHE["nc"] = nc
    return nc


def run(queries, keys, values, trace=False, **kw):
    """Run the SPMD kernel; returns (out_full, BassKernelResults)."""
    nc = _build()
    # [B, L, H, E] -> heads-major layouts the device DMAs straight in.
    qh = np.transpose(np.asarray(queries), (0, 2, 3, 1)).reshape(B * H, E, L)
    qh = np.ascontiguousarray(qh).astype(ml_dtypes.bfloat16)   # [64, E, L]
    kh = np.transpose(np.asarray(keys), (0, 2, 3, 1)).reshape(B * H, E, L)
    # [64, E, L] -> [64, 2, 8, E, 128]: half h gets s-chunks 2c+h
    kh = kh.reshape(B * H, E, NCHUNK // 2, 2, P)
    kh = np.ascontiguousarray(np.transpose(kh, (0, 3, 2, 1, 4)))
    kh = kh.astype(ml_dtypes.bfloat16)
    vh = np.transpose(np.asarray(values), (0, 2, 1, 3)).reshape(B * H, L, E)
    vh = np.concatenate([vh, np.ones((B * H, L, 1), np.float32)], axis=2)
    vh = np.ascontiguousarray(vh).astype(ml_dtypes.bfloat16)
    in_maps = [
        {"qt": qh[c * PAIRS : (c + 1) * PAIRS],
         "kt": kh[c * PAIRS : (c + 1) * PAIRS],
         "v": vh[c * PAIRS : (c + 1) * PAIRS]}
        for c in range(N_CORES)
    ]
    res = run_bass_kernel_spmd(nc, in_maps, list(range(N_CORES)),
                               trace=trace, **kw)
    # [64, E+1, L]: rows 0..63 unnormalized O^T, row 64 the softmax sums
    oh = np.concatenate([res.results[c]["o"] for c in range(N_CORES)], axis=0)
    onorm = oh[:, 0:E, :] / oh[:, E : E + 1, :]          # softmax divide
    out = np.transpose(onorm.reshape(B, H, E, L), (0, 3, 1, 2))
    return np.ascontiguousarray(out), res


def kernel(queries, keys, values):
    out, _ = run(queries, keys, values)
    return out
